# revision 40
# baseline (speedup 1.0000x reference)
"""FP8 batch-matmul-dense kernel for Trainium2 (8 NeuronCores, batch-sharded).

Problem: out[b] = fp8qdq(x)[b] @ fp8qdq(w)[b] + bias[b]
  x: [32, 512, 2048] f32, w: [32, 2048, 2048] f32, bias: [32, 1, 2048] f32
  fp8qdq = torchao-style dynamic tensorwise scaling: s = 448/amax(|t|),
  q = e4m3fn(t*s), dq = q/s. Global (whole-tensor) amax.

Sharding: batch axis across 8 cores, 4 slices each (expert-parallel style).

v3 design (single fused NEFF):
  Phase A streams x then w at fp32, computing exact local amaxes on DVE;
  amax_x / amax_w are AllReduce(max)'d (a dummy warmup AllReduce pays the
  first-collective setup under the x loads). x is PE-transposed as it
  arrives and drained to a RAW fp16 xT (8MiB, no scale needed) so the
  transposes never gate on the ARx result; once sx lands, ACT quantizes
  xT -> 4MiB resident fp8 lhsT codes and xT's space is recycled. The tail
  of the w stream (last RETAIN row-pair tiles in stream order) is
  ACT-downcast to resident fp16 (1MiB/tile), cutting the phase-B re-read
  by 2MiB/tile; the stream order is permuted so the retained set spreads
  across batches b1..b3, balancing phase-B DMA per batch against the PE.
  Phase B re-reads only the non-retained w, quantizes on DVE (fp32 for
  re-read tiles, 2x-rate fp16 for retained), and runs DoubleRow fp8
  matmuls (fp32 PSUM accum) in mt-pair sweeps over 8 PSUM banks, drains
  bias+rescale to bf16 and stores via SWDGE (host upcasts).

Performance model (from ntff profiling):
  - The 16 SDMA engines (~22GB/s each on 8-16KB descriptors) bind phase A
    (80MiB: 16 x + 64 w) and roughly tie the PE in phase B (re-read
    50MiB + 8 out vs ~160us of DoubleRow matmul). All tiles move as
    [128, 2, N] row-pairs (one 16KB-contiguous descriptor per partition).
  - Engine queues are strict FIFO: all load triggers ride the sync HWDGE
    ring; the scalar (ACT) queue holds only the x drains / xqt quants /
    retention downcasts, each gated strictly later than the last, so
    nothing head-of-line blocks. sx math sits SX_DEPTH w-reduces deep in
    the DVE FIFO so DVE reaches it just as the ARx result lands.
  - The ARw collective (~40us against a busy SDMA path) is covered by a
    4-deep re-read prefetch prologue into the freed stage slots.

Quantization math (matches the reference lattice exactly): s' = 224/amax
  (= fl(448/amax)/2 exactly) because TRN fp8_e4m3 tops out at 240, not
  448: the OCP e4m3fn lattice scaled by 1/2 lands exactly on the TRN
  lattice. Matmul runs on raw fp8 codes (exact products, fp32 PSUM
  accum); output is rescaled by c = 1/(sx'*sw'). x codes pass through a
  raw fp16 intermediate and retained w tiles are quantized from fp16:
  the extra 2^-11 rounding flips ~0.8% of codes by 1 ulp, adding ~1e-2
  of the 2e-2 relative budget (measured: comfortably inside the gate).

Per-core HBM traffic: 16 (x) + 64 (w) + 50 (w re-read) + 8 (out bf16)
= 138MiB, one NEFF ramp.
"""

import os
import sys

for _p in ("/root/.axon_site", "/root/.axon_site/_ro/trn_rl_repo", "/opt/trn_rl_repo"):
    if os.path.isdir(_p) and _p not in sys.path:
        sys.path.append(_p)

import numpy as np

import concourse.bass as bass
import concourse.bass_isa as bass_isa
import concourse.mybir as mybir
import concourse.tile as tile
from concourse import bacc
from concourse.bass_utils import run_bass_kernel_spmd
from concourse.masks import make_identity

# Problem shape (hardcoded per contest rules).
B, M, K, N = 32, 512, 2048, 2048
NCORES = 8
BL = B // NCORES          # 4 batch slices per core
P = 128
KT = K // P               # 16 k-tiles per batch
KP = KT // 2              # 8 k-groups (256 rows, row-pair packed) per batch
MT = M // P               # 4 m-tiles
NFREE = 512               # matmul moving free dim (one PSUM bank)
NT = N // NFREE           # 4 n-tiles
SX_DEPTH = 14             # staged (2MiB) w reduces before sx in the DVE FIFO
RETAIN = 7                # w k-group tiles retained as fp16 (with_bias: -2)
PREFETCH = 3              # phase-B re-read loads in flight before 1st quant
FP8_HALF_MAX = 224.0      # 448/2: OCP grid mapped onto TRN e4m3

F32 = mybir.dt.float32
F16 = mybir.dt.float16
BF16 = mybir.dt.bfloat16
FP8 = mybir.dt.float8e4

_cache = {}


def _build_fused_nc(with_bias=True):
    nc = bacc.Bacc("TRN2", target_bir_lowering=False, debug=False, num_devices=NCORES)
    x = nc.dram_tensor("x", [BL, M, K], F32, kind="ExternalInput")
    w = nc.dram_tensor("w", [BL, K, N], F32, kind="ExternalInput")
    bias = nc.dram_tensor("bias", [BL, 1, N], F32, kind="ExternalInput")
    consts = nc.dram_tensor("consts", [1, 2], F32, kind="ExternalInput")
    out = nc.dram_tensor("out", [BL, M, N], BF16, kind="ExternalOutput")

    rg = [list(range(NCORES))]
    retain = RETAIN if with_bias is False else RETAIN - 2

    # w stream order: natural order with the retained set moved to the
    # end so retention only needs SBUF after the xT space frees. The
    # retained set spreads over b1..b3 to even phase-B DMA per batch.
    flat = [(b, t) for b in range(BL) for t in range(KP)]
    # spread across batches so every batch keeps ~38us of phase-B re-read
    # DMA to overlap its PE sweeps (b3-heavy retention leaves a pure-
    # compute tail instead)
    ret_set = [(0, 7), (1, 6), (1, 7), (2, 6),
               (2, 7), (3, 6), (3, 7)][-retain:]
    stream_plan = [bt for bt in flat if bt not in ret_set] + ret_set

    def w_pair_src(b, t):
        """w[b] rows [256t, 256t+256) as [128, 2, N]: partition p holds DRAM
        rows 2p/2p+1 -> one 16KB-contiguous descriptor per partition."""
        return w[b, t * 2 * P:(t + 1) * 2 * P, :].rearrange(
            "(p r) n -> p r n", r=2
        )

    def x_half_src(b, s):
        """x[b] rows [256s, 256s+256) as [128, 2, K]: partition p holds
        rows {256s + p, 256s + 128 + p} (plain m-blocks u = 2s, 2s+1)."""
        return x[b, s * 2 * P:(s + 1) * 2 * P, :].rearrange(
            "(u p) n -> p u n", u=2
        )

    with tile.TileContext(nc) as tc:
        with (
            tc.tile_pool(name="small", bufs=1) as small,
            tc.tile_pool(name="acc", bufs=1) as accp,
            tc.tile_pool(name="xqt", bufs=1) as xqtp,
            tc.tile_pool(name="wstage", bufs=3) as wstage,
            tc.tile_pool(name="dram", bufs=6, space="DRAM") as dram,
        ):
            ident = small.tile([P, P], F32, name="ident")
            make_identity(nc, ident[:])
            cst = small.tile([1, 2], F32, name="cst")
            nc.sync.dma_start(cst[:], consts[0:1, :])
            # scl slots: 0=1/ax, 1=sx, 2=1/aw, 3=sw, 4=sx*sw, 5=c
            scl = small.tile([1, 8], F32, name="scl")
            axg = small.tile([1, 1], F32, name="axg")
            awg = small.tile([1, 1], F32, name="awg")
            cb = small.tile([P, 4], F32, name="cb")   # 0=sx, 1=sw, 2=c

            acc = accp.tile([P, 8 + BL * KP], F32, name="acc")
            red = accp.tile([P, 2], F32, name="red")
            par = accp.tile([P, 2], F32, name="par")

            # resident fp8 lhsT codes, batch-major so each batch's quant is
            # one contiguous DVE op: [ki, b, t, par, u*128 + c] with
            # (ki, par) pairing k = 256t + 2*ki + par (matches w pairing)
            # and plain m-blocks m = u*128 + c.
            xqt = xqtp.tile([P, BL, KP, 2, M], FP8, name="xqt")

            dum_in = dram.tile([1, 8], F32, name="dum_in")
            dum_out = dram.tile([1, 8], F32, name="dum_out")
            dum2_in = dram.tile([1, 8], F32, name="dum2_in")
            dum2_out = dram.tile([1, 8], F32, name="dum2_out")
            ar_in = dram.tile([1, 8], F32, name="ar_in")
            ar_out = dram.tile([1, 8], F32, name="ar_out")

            # warmup collective: pays the ~80us first-collective setup while
            # the x loads stream.
            nc.gpsimd.dma_start(dum_in[0:1, 0:2], cst[:])
            nc.gpsimd.collective_compute(
                "AllReduce", mybir.AluOpType.max, replica_groups=rg,
                ins=[dum_in.opt()], outs=[dum_out.opt()],
            )

            # xT lives on the RIGHT SBUF stack so its release (gated on the
            # ARx-dependent xqt quants, which run late, hidden under the
            # ARw collective) never blocks the retention pool, which takes
            # xstage's LEFT-stack space as soon as the transposes finish.
            xtp = tc.alloc_tile_pool(name="xt", bufs=BL, side="right")
            xstage = tc.alloc_tile_pool(name="xstage", bufs=3)
            trps = tc.alloc_tile_pool(name="trps", bufs=6, space="PSUM")

            col = [8]
            wret = {}
            wretp = [None]
            nld = [0]

            def stage_w_load(bt):
                # alternate the two HWDGE rings (sync/scalar): a single
                # ring's in-order completion handling costs ~0.6us/tile.
                # (sync also carries the transpose-gated x loads up front,
                # so early scalar-ring w tiles keep the DMA fed.)
                nld[0] += 1
                eng = nc.sync if nld[0] % 2 == 0 else nc.scalar
                ws = wstage.tile([P, 2, N], F32, name="ws", tag="ws")
                eng.dma_start(ws[:], w_pair_src(*bt))
                nc.vector.tensor_reduce(
                    acc[:, col[0]:col[0] + 1], ws[:],
                    axis=mybir.AxisListType.XY, op=mybir.AluOpType.max,
                    apply_absolute_value=True,
                )
                col[0] += 1
                if bt in ret_set:
                    wr = wretp[0].tile([P, 2, N], F16, name="wr", tag="wr")
                    nc.scalar.activation(
                        wr[:], ws[:], mybir.ActivationFunctionType.Copy,
                    )
                    wret[bt] = wr

            # ---- x: stream, amax, PE-transpose, drain raw fp16 xT ----
            # The x stream is transpose-paced (~20us/batch); w-load blocks
            # interleave between x batches so the stream keeps the DMA
            # engines saturated while x trickles.
            xts = []
            for b in range(BL):
                views = {}
                for s in range(2):
                    st = xstage.tile([P, 2, K], F32, name="xs", tag="xs")
                    nc.sync.dma_start(st[:], x_half_src(b, s))
                    nc.vector.tensor_reduce(
                        acc[:, 2 * b + s:2 * b + s + 1], st[:],
                        axis=mybir.AxisListType.XY, op=mybir.AluOpType.max,
                        apply_absolute_value=True,
                    )
                    for j in range(2):
                        views[2 * s + j] = st[:, j, :].rearrange(
                            "p (k two) -> p two k", two=2
                        )
                xt = xtp.tile([P, KP, 2, M], F16, name="xt", tag="xt")
                for t in range(KP):
                    for parp in range(2):
                        # one single-bank [P, 512] psum per (t, parp):
                        # a 2-bank psum AP puts the ACT drain on a slow
                        # cross-bank read path (~4x)
                        ps = trps.tile([P, M], F32, name="tps", tag="tps")
                        for u in range(MT):
                            nc.tensor.transpose(
                                ps[:, u * P:(u + 1) * P],
                                views[u][:, parp, t * P:(t + 1) * P],
                                ident[:],
                            )
                        nc.scalar.activation(
                            xt[:, t, parp, :], ps[:],
                            mybir.ActivationFunctionType.Copy,
                        )
                xts.append(xt)
                if b < BL - 1:
                    for bt in stream_plan[8 * b:8 * (b + 1)]:
                        stage_w_load(bt)
                if b == 1:
                    # mid-stream alignment collective: re-synchronizes the
                    # 8 cores' CC paths while the stream hides its flight,
                    # so the real AllReduce at the phase boundary doesn't
                    # pay accumulated inter-core skew.
                    nc.gpsimd.dma_start(dum2_in[0:1, 0:2], cst[:])
                    nc.gpsimd.collective_compute(
                        "AllReduce", mybir.AluOpType.max, replica_groups=rg,
                        ins=[dum2_in.opt()], outs=[dum2_out.opt()],
                    )

            trps.release()
            xstage.release()

            # retention pool over xstage's freed space (gated only on the
            # x transposes, NOT on the ARx result — the x amaxes and the
            # collective can land arbitrarily late without stalling the w
            # stream or the retention copies)
            wretp[0] = tc.alloc_tile_pool(name="wret", bufs=max(retain, 1))

            for bt in stream_plan[8 * (BL - 1):]:
                stage_w_load(bt)

            # ---- phase A -> B boundary: ONE fused AllReduce ----
            # A single 2-element AllReduce(max) carries [amax_x, amax_w].
            # Separate ARx/ARw collectives serialize on the CC cores: a
            # straggling ARx (exec latency varies 25-75us with inter-core
            # skew) would delay ARw far past the stream end.
            nc.vector.tensor_reduce(
                red[:, 0:1], acc[:, 0:2 * BL],
                axis=mybir.AxisListType.X, op=mybir.AluOpType.max,
            )
            nc.vector.tensor_reduce(
                red[:, 1:2], acc[:, 8:col[0]],
                axis=mybir.AxisListType.X, op=mybir.AluOpType.max,
            )
            nc.gpsimd.partition_all_reduce(
                par[:, 0:2], red[:, 0:2], channels=P,
                reduce_op=bass_isa.ReduceOp.max,
            )
            nc.gpsimd.dma_start(ar_in[0:1, 0:2], par[0:1, 0:2])
            nc.gpsimd.collective_compute(
                "AllReduce", mybir.AluOpType.max, replica_groups=rg,
                ins=[ar_in.opt()], outs=[ar_out.opt()],
            )
            nc.gpsimd.dma_start(axg[:], ar_out[0:1, 0:1])
            nc.gpsimd.dma_start(awg[:], ar_out[0:1, 1:2])
            # sx = 224 / max(amax_x, 1e-12); sw likewise; c = 1/(sx*sw)
            nc.vector.tensor_scalar_max(axg[:], axg[:], 1e-12)
            nc.vector.reciprocal(scl[0:1, 0:1], axg[:])
            nc.vector.tensor_scalar_mul(scl[0:1, 1:2], scl[0:1, 0:1], FP8_HALF_MAX)
            nc.gpsimd.partition_broadcast(cb[:, 0:1], scl[0:1, 1:2])
            sx_ap = cb[:, 0:1]
            nc.vector.tensor_scalar_max(awg[:], awg[:], 1e-12)
            nc.vector.reciprocal(scl[0:1, 2:3], awg[:])
            nc.vector.tensor_scalar_mul(scl[0:1, 3:4], scl[0:1, 2:3], FP8_HALF_MAX)
            nc.vector.tensor_tensor(
                scl[0:1, 4:5], scl[0:1, 1:2], scl[0:1, 3:4],
                mybir.AluOpType.mult,
            )
            nc.vector.reciprocal(scl[0:1, 5:6], scl[0:1, 4:5])
            nc.gpsimd.partition_broadcast(cb[:, 1:2], scl[0:1, 3:4])
            nc.gpsimd.partition_broadcast(cb[:, 2:3], scl[0:1, 5:6])
            sw_ap = cb[:, 1:2]
            c_ap = cb[:, 2:3]

            # xqt quants on DVE right behind the scale math
            for b in range(BL):
                nc.vector.tensor_scalar(
                    xqt[:, b], xts[b][:], sx_ap, None,
                    op0=mybir.AluOpType.mult,
                )
            xtp.release()

            # ---- phase B: software-pipelined re-read + quantize + mm ----
            # Engine split keeps every FIFO stall-free: DVE runs ONLY the
            # re-read quants (so the load pipeline is never queued behind
            # drains at batch boundaries); ACT runs the retained-tile
            # quants (fp16, ready as soon as sw lands) plus all drains.
            # Matmuls sweep u-granular (4 PSUM banks), ping-ponged so the
            # PE never waits on a bank drain.
            wqp = tc.alloc_tile_pool(name="wq", bufs=11)
            ostp = tc.alloc_tile_pool(name="ost", bufs=2)
            if with_bias:
                bias1p = tc.alloc_tile_pool(name="bias1", bufs=1)
                biasbp = tc.alloc_tile_pool(name="biasb", bufs=2)

            reread_plan = [bt for bt in flat if bt not in ret_set]
            stage_tiles = {}
            nload = [0]

            def issue_load():
                # prologue rides sync only (the gpsimd queue may be held
                # by the in-flight ARw collective); the steady pipeline
                # alternates sync/gpsimd (the scalar queue is busy with
                # drains, which would head-of-line block triggers).
                if nload[0] >= len(reread_plan):
                    return
                bt = reread_plan[nload[0]]
                eng = nc.sync if (nload[0] < PREFETCH or nload[0] % 2 == 0) \
                    else nc.gpsimd
                st = wstage.tile([P, 2, N], F32, name="ws", tag="ws")
                eng.dma_start(st[:], w_pair_src(*bt))
                stage_tiles[bt] = st
                nload[0] += 1

            for _ in range(PREFETCH):
                issue_load()

            mmps = tc.alloc_tile_pool(name="mmps", bufs=4, space="PSUM")

            wq_all = {}
            for b_, t_ in flat:
                wqt = wqp.tile([P, 2, N], FP8, name="wq", tag="wq")
                if (b_, t_) in ret_set:
                    if with_bias:
                        nc.vector.tensor_scalar(
                            wqt[:], wret[(b_, t_)][:], sw_ap, None,
                            op0=mybir.AluOpType.mult,
                        )
                    else:
                        nc.scalar.activation(
                            wqt[:], wret[(b_, t_)][:],
                            mybir.ActivationFunctionType.Copy, scale=sw_ap,
                        )
                else:
                    nc.vector.tensor_scalar(
                        wqt[:], stage_tiles.pop((b_, t_))[:], sw_ap, None,
                        op0=mybir.AluOpType.mult,
                    )
                    issue_load()
                wq_all[(b_, t_)] = wqt

                if t_ == KP - 1:
                    b = b_
                    if with_bias:
                        b1 = bias1p.tile([1, N], BF16, name="b1", tag="b1")
                        nc.gpsimd.dma_start(b1[:], bias[b, :, :])
                        bb = biasbp.tile([P, N], BF16, name="bb", tag="bb")
                        nc.gpsimd.partition_broadcast(bb[:], b1[:])

                    wq_tiles = [wq_all.pop((b, t)) for t in range(KP)]
                    ost2 = None
                    for u in range(MT):
                        if u % 2 == 0:
                            ost2 = ostp.tile([P, 2, N], BF16,
                                             name="ost", tag="ost")
                        psums = [
                            mmps.tile([P, 2 * NFREE], F32,
                                      name=f"mm{h}", tag="mm")
                            for h in range(NT // 2)
                        ]
                        for t in range(KP):
                            lhsT = xqt[:, b, t, :, u * P:(u + 1) * P]
                            for nt in range(NT):
                                ps = psums[nt // 2]
                                lo = (nt % 2) * NFREE
                                nc.tensor.matmul(
                                    ps[:, lo:lo + NFREE],
                                    lhsT,
                                    wq_tiles[t][:, :,
                                                nt * NFREE:(nt + 1) * NFREE],
                                    start=(t == 0),
                                    stop=(t == KP - 1),
                                    perf_mode=mybir.MatmulPerfMode.DoubleRow,
                                )
                        for h in range(NT // 2):
                            o_ap = ost2[:, u % 2,
                                        h * 2 * NFREE:(h + 1) * 2 * NFREE]
                            if with_bias:
                                nc.vector.scalar_tensor_tensor(
                                    o_ap, psums[h][:], c_ap,
                                    bb[:, h * 2 * NFREE:(h + 1) * 2 * NFREE],
                                    op0=mybir.AluOpType.mult,
                                    op1=mybir.AluOpType.add,
                                )
                            else:
                                nc.scalar.activation(
                                    o_ap, psums[h][:],
                                    mybir.ActivationFunctionType.Copy,
                                    scale=c_ap,
                                )
                        if u % 2 == 1:
                            # plain m-blocks: m = 256*mh + mi*128 + c; the
                            # store rides the scalar HWDGE ring right after
                            # its drains (sync/gpsimd carry the re-reads).
                            mh = u // 2
                            dst = out[b, 2 * mh * P:(2 * mh + 2) * P,
                                      :].rearrange("(r p) n -> p r n", r=2)
                            nc.scalar.dma_start(dst, ost2[:])

            mmps.release()
            if with_bias:
                biasbp.release()
                bias1p.release()
            ostp.release()
            wqp.release()
            wretp[0].release()

    nc.compile()
    return nc


def _get_nc(with_bias):
    key = "fused_b" if with_bias else "fused_nb"
    if key not in _cache:
        _cache[key] = _build_fused_nc(with_bias)
    return _cache[key]


# test.py introspection: exec times (ns) of the last kernel() call.
last_run_info = {}


def kernel(input, weight, bias, _profile=False, _repeat=1, _trace_kwargs=None):
    input = np.ascontiguousarray(input, dtype=np.float32)
    weight = np.ascontiguousarray(weight, dtype=np.float32)
    bias = np.ascontiguousarray(bias, dtype=np.float32)
    assert input.shape == (B, M, K) and weight.shape == (B, K, N)
    assert bias.shape == (B, 1, N)

    consts = np.array([[FP8_HALF_MAX, 1.0]], dtype=np.float32)
    in_maps = [
        {
            "x": input[c * BL:(c + 1) * BL],
            "w": weight[c * BL:(c + 1) * BL],
            "bias": bias[c * BL:(c + 1) * BL],
            "consts": consts,
        }
        for c in range(NCORES)
    ]

    kw = dict(trace=_profile)
    if _trace_kwargs:
        kw.update(_trace_kwargs)

    # bias is exactly zero in this workload; the no-bias NEFF skips the
    # broadcast-add (drains become scaled copies, ACT-assisted at the tail).
    # The with-bias NEFF stays available for correctness on any input.
    nc = _get_nc(with_bias=bool(np.any(bias)))
    times = []
    res = None
    for _ in range(max(1, _repeat)):
        res = run_bass_kernel_spmd(nc, in_maps, core_ids=list(range(NCORES)), **kw)
        times.append(res.exec_time_ns)

    last_run_info.clear()
    last_run_info["amax_times"] = None
    last_run_info["mm_times"] = times
    last_run_info["amax_exec_ns"] = None
    last_run_info["mm_exec_ns"] = min(t for t in times if t) if any(times) else None
    last_run_info["mm_results"] = res

    out = np.concatenate(
        [np.asarray(res.results[c]["out"]).astype(np.float32) for c in range(NCORES)],
        axis=0,
    )
    return out


# revision 42
# speedup vs baseline: 1.0302x; 1.0302x over previous
"""FP8 batch-matmul-dense kernel for Trainium2 (8 NeuronCores, batch-sharded).

Problem: out[b] = fp8qdq(x)[b] @ fp8qdq(w)[b] + bias[b]
  x: [32, 512, 2048] f32, w: [32, 2048, 2048] f32, bias: [32, 1, 2048] f32
  fp8qdq = torchao-style dynamic tensorwise scaling: s = 448/amax(|t|),
  q = e4m3fn(t*s), dq = q/s. Global (whole-tensor) amax.

Sharding: batch axis across 8 cores, 4 slices each (expert-parallel style).

v3 design (single fused NEFF):
  Phase A streams x then w at fp32, computing exact local amaxes on DVE;
  amax_x / amax_w are AllReduce(max)'d (a dummy warmup AllReduce pays the
  first-collective setup under the x loads). x is PE-transposed as it
  arrives and drained to a RAW fp16 xT (8MiB, no scale needed) so the
  transposes never gate on the ARx result; once sx lands, ACT quantizes
  xT -> 4MiB resident fp8 lhsT codes and xT's space is recycled. The tail
  of the w stream (last RETAIN row-pair tiles in stream order) is
  ACT-downcast to resident fp16 (1MiB/tile), cutting the phase-B re-read
  by 2MiB/tile; the stream order is permuted so the retained set spreads
  across batches b1..b3, balancing phase-B DMA per batch against the PE.
  Phase B re-reads only the non-retained w, quantizes on DVE (fp32 for
  re-read tiles, 2x-rate fp16 for retained), and runs DoubleRow fp8
  matmuls (fp32 PSUM accum) in mt-pair sweeps over 8 PSUM banks, drains
  bias+rescale to bf16 and stores via SWDGE (host upcasts).

Performance model (from ntff profiling):
  - The 16 SDMA engines (~22GB/s each on 8-16KB descriptors) bind phase A
    (80MiB: 16 x + 64 w) and roughly tie the PE in phase B (re-read
    50MiB + 8 out vs ~160us of DoubleRow matmul). All tiles move as
    [128, 2, N] row-pairs (one 16KB-contiguous descriptor per partition).
  - Engine queues are strict FIFO: all load triggers ride the sync HWDGE
    ring; the scalar (ACT) queue holds only the x drains / xqt quants /
    retention downcasts, each gated strictly later than the last, so
    nothing head-of-line blocks. sx math sits SX_DEPTH w-reduces deep in
    the DVE FIFO so DVE reaches it just as the ARx result lands.
  - The ARw collective (~40us against a busy SDMA path) is covered by a
    4-deep re-read prefetch prologue into the freed stage slots.

Quantization math (matches the reference lattice exactly): s' = 224/amax
  (= fl(448/amax)/2 exactly) because TRN fp8_e4m3 tops out at 240, not
  448: the OCP e4m3fn lattice scaled by 1/2 lands exactly on the TRN
  lattice. Matmul runs on raw fp8 codes (exact products, fp32 PSUM
  accum); output is rescaled by c = 1/(sx'*sw'). x codes pass through a
  raw fp16 intermediate and retained w tiles are quantized from fp16:
  the extra 2^-11 rounding flips ~0.8% of codes by 1 ulp, adding ~1e-2
  of the 2e-2 relative budget (measured: comfortably inside the gate).

Per-core HBM traffic: 16 (x) + 64 (w) + 50 (w re-read) + 8 (out bf16)
= 138MiB, one NEFF ramp.
"""

import os
import sys

for _p in ("/root/.axon_site", "/root/.axon_site/_ro/trn_rl_repo", "/opt/trn_rl_repo"):
    if os.path.isdir(_p) and _p not in sys.path:
        sys.path.append(_p)

import numpy as np

import concourse.bass as bass
import concourse.bass_isa as bass_isa
import concourse.mybir as mybir
import concourse.tile as tile
from concourse import bacc
from concourse.bass_utils import run_bass_kernel_spmd
from concourse.masks import make_identity

# Problem shape (hardcoded per contest rules).
B, M, K, N = 32, 512, 2048, 2048
NCORES = 8
BL = B // NCORES          # 4 batch slices per core
P = 128
KT = K // P               # 16 k-tiles per batch
KP = KT // 2              # 8 k-groups (256 rows, row-pair packed) per batch
MT = M // P               # 4 m-tiles
NFREE = 512               # matmul moving free dim (one PSUM bank)
NT = N // NFREE           # 4 n-tiles
SX_DEPTH = 14             # staged (2MiB) w reduces before sx in the DVE FIFO
RETAIN = 7                # w k-group tiles retained as fp16 (with_bias: -2)
PREFETCH = 3              # phase-B re-read loads in flight before 1st quant
FP8_HALF_MAX = 224.0      # 448/2: OCP grid mapped onto TRN e4m3

F32 = mybir.dt.float32
F16 = mybir.dt.float16
BF16 = mybir.dt.bfloat16
FP8 = mybir.dt.float8e4

_cache = {}


def _build_fused_nc(with_bias=True):
    nc = bacc.Bacc("TRN2", target_bir_lowering=False, debug=False, num_devices=NCORES)
    x = nc.dram_tensor("x", [BL, M, K], F32, kind="ExternalInput")
    w = nc.dram_tensor("w", [BL, K, N], F32, kind="ExternalInput")
    bias = nc.dram_tensor("bias", [BL, 1, N], F32, kind="ExternalInput")
    consts = nc.dram_tensor("consts", [1, 2], F32, kind="ExternalInput")
    out = nc.dram_tensor("out", [BL, M, N], BF16, kind="ExternalOutput")

    rg = [list(range(NCORES))]
    retain = RETAIN if with_bias is False else RETAIN - 2

    # w stream order: natural order with the retained set moved to the
    # end so retention only needs SBUF after the xT space frees. The
    # retained set spreads over b1..b3 to even phase-B DMA per batch.
    flat = [(b, t) for b in range(BL) for t in range(KP)]
    # spread across batches so every batch keeps ~38us of phase-B re-read
    # DMA to overlap its PE sweeps (b3-heavy retention leaves a pure-
    # compute tail instead)
    ret_set = [(0, 7), (1, 6), (1, 7), (2, 6),
               (2, 7), (3, 6), (3, 7)][-retain:]
    stream_plan = [bt for bt in flat if bt not in ret_set] + ret_set

    def w_pair_src(b, t):
        """w[b] rows [256t, 256t+256) as [128, 2, N]: partition p holds DRAM
        rows 2p/2p+1 -> one 16KB-contiguous descriptor per partition."""
        return w[b, t * 2 * P:(t + 1) * 2 * P, :].rearrange(
            "(p r) n -> p r n", r=2
        )

    def x_half_src(b, s):
        """x[b] rows [256s, 256s+256) as [128, 2, K]: partition p holds
        rows {256s + p, 256s + 128 + p} (plain m-blocks u = 2s, 2s+1)."""
        return x[b, s * 2 * P:(s + 1) * 2 * P, :].rearrange(
            "(u p) n -> p u n", u=2
        )

    with tile.TileContext(nc) as tc:
        with (
            tc.tile_pool(name="small", bufs=1) as small,
            tc.tile_pool(name="acc", bufs=1) as accp,
            tc.tile_pool(name="xqt", bufs=1) as xqtp,
            tc.tile_pool(name="wstage", bufs=3) as wstage,
            tc.tile_pool(name="dram", bufs=6, space="DRAM") as dram,
        ):
            ident = small.tile([P, P], F32, name="ident")
            make_identity(nc, ident[:])
            cst = small.tile([1, 2], F32, name="cst")
            nc.sync.dma_start(cst[:], consts[0:1, :])
            # scl slots: 0=1/ax, 1=sx, 2=1/aw, 3=sw, 4=sx*sw, 5=c
            scl = small.tile([1, 8], F32, name="scl")
            axg = small.tile([1, 1], F32, name="axg")
            awg = small.tile([1, 1], F32, name="awg")
            cb = small.tile([P, 4], F32, name="cb")   # 0=sx, 1=sw, 2=c

            acc = accp.tile([P, 8 + BL * KP], F32, name="acc")
            red = accp.tile([P, 2], F32, name="red")
            par = accp.tile([P, 2], F32, name="par")

            # resident fp8 lhsT codes, batch-major so each batch's quant is
            # one contiguous DVE op: [ki, b, t, par, u*128 + c] with
            # (ki, par) pairing k = 256t + 2*ki + par (matches w pairing)
            # and plain m-blocks m = u*128 + c.
            xqt = xqtp.tile([P, BL, KP, 2, M], FP8, name="xqt")

            dum_in = dram.tile([1, 8], F32, name="dum_in")
            dum_out = dram.tile([1, 8], F32, name="dum_out")
            dum2_in = dram.tile([1, 8], F32, name="dum2_in")
            dum2_out = dram.tile([1, 8], F32, name="dum2_out")
            ar_in = dram.tile([1, 8], F32, name="ar_in")
            ar_out = dram.tile([1, 8], F32, name="ar_out")

            # warmup collective: pays the ~80us first-collective setup while
            # the x loads stream.
            nc.gpsimd.dma_start(dum_in[0:1, 0:2], cst[:])
            nc.gpsimd.collective_compute(
                "AllReduce", mybir.AluOpType.max, replica_groups=rg,
                ins=[dum_in.opt()], outs=[dum_out.opt()],
            )

            # xT lives on the RIGHT SBUF stack so its release (gated on the
            # ARx-dependent xqt quants, which run late, hidden under the
            # ARw collective) never blocks the retention pool, which takes
            # xstage's LEFT-stack space as soon as the transposes finish.
            xtp = tc.alloc_tile_pool(name="xt", bufs=BL, side="right")
            xstage = tc.alloc_tile_pool(name="xstage", bufs=3)
            trps = tc.alloc_tile_pool(name="trps", bufs=6, space="PSUM")

            col = [8]
            wret = {}
            wretp = [None]
            nld = [0]

            def stage_w_load(bt):
                # alternate the two HWDGE rings (sync/scalar): a single
                # ring's in-order completion handling costs ~0.6us/tile.
                # (sync also carries the transpose-gated x loads up front,
                # so early scalar-ring w tiles keep the DMA fed.)
                nld[0] += 1
                eng = nc.sync if nld[0] % 2 == 0 else nc.scalar
                ws = wstage.tile([P, 2, N], F32, name="ws", tag="ws")
                eng.dma_start(ws[:], w_pair_src(*bt))
                nc.vector.tensor_reduce(
                    acc[:, col[0]:col[0] + 1], ws[:],
                    axis=mybir.AxisListType.XY, op=mybir.AluOpType.max,
                    apply_absolute_value=True,
                )
                col[0] += 1
                if bt in ret_set:
                    wr = wretp[0].tile([P, 2, N], F16, name="wr", tag="wr")
                    nc.scalar.activation(
                        wr[:], ws[:], mybir.ActivationFunctionType.Copy,
                    )
                    wret[bt] = wr

            # ---- x: stream, amax, PE-transpose, drain raw fp16 xT ----
            # The x stream is transpose-paced (~20us/batch); w-load blocks
            # interleave between x batches so the stream keeps the DMA
            # engines saturated while x trickles.
            xts = []
            for b in range(BL):
                views = {}
                for s in range(2):
                    st = xstage.tile([P, 2, K], F32, name="xs", tag="xs")
                    nc.sync.dma_start(st[:], x_half_src(b, s))
                    nc.vector.tensor_reduce(
                        acc[:, 2 * b + s:2 * b + s + 1], st[:],
                        axis=mybir.AxisListType.XY, op=mybir.AluOpType.max,
                        apply_absolute_value=True,
                    )
                    for j in range(2):
                        views[2 * s + j] = st[:, j, :].rearrange(
                            "p (k two) -> p two k", two=2
                        )
                xt = xtp.tile([P, KP, 2, M], F16, name="xt", tag="xt")
                for t in range(KP):
                    for parp in range(2):
                        # one single-bank [P, 512] psum per (t, parp):
                        # a 2-bank psum AP puts the ACT drain on a slow
                        # cross-bank read path (~4x)
                        ps = trps.tile([P, M], F32, name="tps", tag="tps")
                        for u in range(MT):
                            nc.tensor.transpose(
                                ps[:, u * P:(u + 1) * P],
                                views[u][:, parp, t * P:(t + 1) * P],
                                ident[:],
                            )
                        nc.scalar.activation(
                            xt[:, t, parp, :], ps[:],
                            mybir.ActivationFunctionType.Copy,
                        )
                xts.append(xt)
                if b < BL - 1:
                    for bt in stream_plan[8 * b:8 * (b + 1)]:
                        stage_w_load(bt)

            # ---- amax_x AllReduce trigger (result consumed later) ----
            nc.vector.tensor_reduce(
                red[:, 0:1], acc[:, 0:2 * BL],
                axis=mybir.AxisListType.X, op=mybir.AluOpType.max,
            )
            nc.gpsimd.partition_all_reduce(
                par[:, 0:1], red[:, 0:1], channels=P,
                reduce_op=bass_isa.ReduceOp.max,
            )
            nc.gpsimd.dma_start(ar_in[0:1, 0:1], par[0:1, 0:1])
            nc.gpsimd.collective_compute(
                "AllReduce", mybir.AluOpType.max, replica_groups=rg,
                ins=[ar_in.opt()], outs=[ar_out.opt()],
            )
            nc.gpsimd.dma_start(axg[:], ar_out[0:1, 0:1])

            trps.release()
            xstage.release()

            # retention pool over xstage's freed space (gated only on the
            # x transposes, NOT on the ARx result — the x amaxes and the
            # collective can land arbitrarily late without stalling the w
            # stream or the retention copies)
            wretp[0] = tc.alloc_tile_pool(name="wret", bufs=max(retain, 1))

            for bt in stream_plan[8 * (BL - 1):]:
                stage_w_load(bt)

            # ---- phase A -> B boundary ----
            # DVE FIFO from here: red1 col-reduce, sx math, xqt quants,
            # sw math. The ARx result has long landed, so sx and the xqt
            # quants run immediately after the last w reduce — hidden
            # entirely under the ARw collective's flight time.
            nc.vector.tensor_reduce(
                red[:, 1:2], acc[:, 8:col[0]],
                axis=mybir.AxisListType.X, op=mybir.AluOpType.max,
            )
            # sx = 224 / max(amax_x, 1e-12)
            nc.vector.tensor_scalar_max(axg[:], axg[:], 1e-12)
            nc.vector.reciprocal(scl[0:1, 0:1], axg[:])
            nc.vector.tensor_scalar_mul(scl[0:1, 1:2], scl[0:1, 0:1], FP8_HALF_MAX)
            nc.gpsimd.partition_broadcast(cb[:, 0:1], scl[0:1, 1:2])
            sx_ap = cb[:, 0:1]
            # ARw trigger (the gpsimd broadcast above is ready first, so
            # it never head-of-line blocks these hops)
            nc.gpsimd.partition_all_reduce(
                par[:, 1:2], red[:, 1:2], channels=P,
                reduce_op=bass_isa.ReduceOp.max,
            )
            nc.gpsimd.dma_start(dum2_in[0:1, 0:1], par[0:1, 1:2])
            nc.gpsimd.collective_compute(
                "AllReduce", mybir.AluOpType.max, replica_groups=rg,
                ins=[dum2_in.opt()], outs=[dum2_out.opt()],
            )
            # xqt quants on DVE while the ARw collective flies
            for b in range(BL):
                nc.vector.tensor_scalar(
                    xqt[:, b], xts[b][:], sx_ap, None,
                    op0=mybir.AluOpType.mult,
                )
            xtp.release()
            nc.gpsimd.dma_start(awg[:], dum2_out[0:1, 0:1])
            # sw = 224 / max(amax_w, 1e-12); c = 1/(sx*sw)
            nc.vector.tensor_scalar_max(awg[:], awg[:], 1e-12)
            nc.vector.reciprocal(scl[0:1, 2:3], awg[:])
            nc.vector.tensor_scalar_mul(scl[0:1, 3:4], scl[0:1, 2:3], FP8_HALF_MAX)
            nc.vector.tensor_tensor(
                scl[0:1, 4:5], scl[0:1, 1:2], scl[0:1, 3:4],
                mybir.AluOpType.mult,
            )
            nc.vector.reciprocal(scl[0:1, 5:6], scl[0:1, 4:5])
            nc.gpsimd.partition_broadcast(cb[:, 1:2], scl[0:1, 3:4])
            nc.gpsimd.partition_broadcast(cb[:, 2:3], scl[0:1, 5:6])
            sw_ap = cb[:, 1:2]
            c_ap = cb[:, 2:3]

            # ---- phase B: software-pipelined re-read + quantize + mm ----
            # Engine split keeps every FIFO stall-free: DVE runs ONLY the
            # re-read quants (so the load pipeline is never queued behind
            # drains at batch boundaries); ACT runs the retained-tile
            # quants (fp16, ready as soon as sw lands) plus all drains.
            # Matmuls sweep u-granular (4 PSUM banks), ping-ponged so the
            # PE never waits on a bank drain.
            wqp = tc.alloc_tile_pool(name="wq", bufs=11)
            ostp = tc.alloc_tile_pool(name="ost", bufs=2)
            if with_bias:
                bias1p = tc.alloc_tile_pool(name="bias1", bufs=1)
                biasbp = tc.alloc_tile_pool(name="biasb", bufs=2)

            reread_plan = [bt for bt in flat if bt not in ret_set]
            stage_tiles = {}
            nload = [0]

            def issue_load():
                # prologue rides sync only (the gpsimd queue may be held
                # by the in-flight ARw collective); the steady pipeline
                # alternates sync/gpsimd (the scalar queue is busy with
                # drains, which would head-of-line block triggers).
                if nload[0] >= len(reread_plan):
                    return
                bt = reread_plan[nload[0]]
                eng = nc.sync if (nload[0] < PREFETCH or nload[0] % 2 == 0) \
                    else nc.gpsimd
                st = wstage.tile([P, 2, N], F32, name="ws", tag="ws")
                eng.dma_start(st[:], w_pair_src(*bt))
                stage_tiles[bt] = st
                nload[0] += 1

            for _ in range(PREFETCH):
                issue_load()

            mmps = tc.alloc_tile_pool(name="mmps", bufs=4, space="PSUM")

            wq_all = {}
            for b_, t_ in flat:
                wqt = wqp.tile([P, 2, N], FP8, name="wq", tag="wq")
                if (b_, t_) in ret_set:
                    if with_bias:
                        nc.vector.tensor_scalar(
                            wqt[:], wret[(b_, t_)][:], sw_ap, None,
                            op0=mybir.AluOpType.mult,
                        )
                    else:
                        nc.scalar.activation(
                            wqt[:], wret[(b_, t_)][:],
                            mybir.ActivationFunctionType.Copy, scale=sw_ap,
                        )
                else:
                    nc.vector.tensor_scalar(
                        wqt[:], stage_tiles.pop((b_, t_))[:], sw_ap, None,
                        op0=mybir.AluOpType.mult,
                    )
                    issue_load()
                wq_all[(b_, t_)] = wqt

                if t_ == KP - 1:
                    b = b_
                    if with_bias:
                        b1 = bias1p.tile([1, N], BF16, name="b1", tag="b1")
                        nc.gpsimd.dma_start(b1[:], bias[b, :, :])
                        bb = biasbp.tile([P, N], BF16, name="bb", tag="bb")
                        nc.gpsimd.partition_broadcast(bb[:], b1[:])

                    wq_tiles = [wq_all.pop((b, t)) for t in range(KP)]
                    ost2 = None
                    for u in range(MT):
                        if u % 2 == 0:
                            ost2 = ostp.tile([P, 2, N], BF16,
                                             name="ost", tag="ost")
                        psums = [
                            mmps.tile([P, 2 * NFREE], F32,
                                      name=f"mm{h}", tag="mm")
                            for h in range(NT // 2)
                        ]
                        for t in range(KP):
                            lhsT = xqt[:, b, t, :, u * P:(u + 1) * P]
                            for nt in range(NT):
                                ps = psums[nt // 2]
                                lo = (nt % 2) * NFREE
                                nc.tensor.matmul(
                                    ps[:, lo:lo + NFREE],
                                    lhsT,
                                    wq_tiles[t][:, :,
                                                nt * NFREE:(nt + 1) * NFREE],
                                    start=(t == 0),
                                    stop=(t == KP - 1),
                                    perf_mode=mybir.MatmulPerfMode.DoubleRow,
                                )
                        for h in range(NT // 2):
                            o_ap = ost2[:, u % 2,
                                        h * 2 * NFREE:(h + 1) * 2 * NFREE]
                            if with_bias:
                                nc.vector.scalar_tensor_tensor(
                                    o_ap, psums[h][:], c_ap,
                                    bb[:, h * 2 * NFREE:(h + 1) * 2 * NFREE],
                                    op0=mybir.AluOpType.mult,
                                    op1=mybir.AluOpType.add,
                                )
                            else:
                                nc.scalar.activation(
                                    o_ap, psums[h][:],
                                    mybir.ActivationFunctionType.Copy,
                                    scale=c_ap,
                                )
                        if u % 2 == 1:
                            # plain m-blocks: m = 256*mh + mi*128 + c; the
                            # store rides the scalar HWDGE ring right after
                            # its drains (sync/gpsimd carry the re-reads).
                            mh = u // 2
                            dst = out[b, 2 * mh * P:(2 * mh + 2) * P,
                                      :].rearrange("(r p) n -> p r n", r=2)
                            nc.scalar.dma_start(dst, ost2[:])

            mmps.release()
            if with_bias:
                biasbp.release()
                bias1p.release()
            ostp.release()
            wqp.release()
            wretp[0].release()

    nc.compile()
    return nc


def _get_nc(with_bias):
    key = "fused_b" if with_bias else "fused_nb"
    if key not in _cache:
        _cache[key] = _build_fused_nc(with_bias)
    return _cache[key]


# test.py introspection: exec times (ns) of the last kernel() call.
last_run_info = {}


def kernel(input, weight, bias, _profile=False, _repeat=1, _trace_kwargs=None):
    input = np.ascontiguousarray(input, dtype=np.float32)
    weight = np.ascontiguousarray(weight, dtype=np.float32)
    bias = np.ascontiguousarray(bias, dtype=np.float32)
    assert input.shape == (B, M, K) and weight.shape == (B, K, N)
    assert bias.shape == (B, 1, N)

    consts = np.array([[FP8_HALF_MAX, 1.0]], dtype=np.float32)
    in_maps = [
        {
            "x": input[c * BL:(c + 1) * BL],
            "w": weight[c * BL:(c + 1) * BL],
            "bias": bias[c * BL:(c + 1) * BL],
            "consts": consts,
        }
        for c in range(NCORES)
    ]

    kw = dict(trace=_profile)
    if _trace_kwargs:
        kw.update(_trace_kwargs)

    # bias is exactly zero in this workload; the no-bias NEFF skips the
    # broadcast-add (drains become scaled copies, ACT-assisted at the tail).
    # The with-bias NEFF stays available for correctness on any input.
    nc = _get_nc(with_bias=bool(np.any(bias)))
    times = []
    res = None
    for _ in range(max(1, _repeat)):
        res = run_bass_kernel_spmd(nc, in_maps, core_ids=list(range(NCORES)), **kw)
        times.append(res.exec_time_ns)

    last_run_info.clear()
    last_run_info["amax_times"] = None
    last_run_info["mm_times"] = times
    last_run_info["amax_exec_ns"] = None
    last_run_info["mm_exec_ns"] = min(t for t in times if t) if any(times) else None
    last_run_info["mm_results"] = res

    out = np.concatenate(
        [np.asarray(res.results[c]["out"]).astype(np.float32) for c in range(NCORES)],
        axis=0,
    )
    return out


# revision 46
# speedup vs baseline: 1.0381x; 1.0077x over previous
"""FP8 batch-matmul-dense kernel for Trainium2 (8 NeuronCores, batch-sharded).

Problem: out[b] = fp8qdq(x)[b] @ fp8qdq(w)[b] + bias[b]
  x: [32, 512, 2048] f32, w: [32, 2048, 2048] f32, bias: [32, 1, 2048] f32
  fp8qdq = torchao-style dynamic tensorwise scaling: s = 448/amax(|t|),
  q = e4m3fn(t*s), dq = q/s. Global (whole-tensor) amax.

Sharding: batch axis across 8 cores, 4 slices each (expert-parallel style).

v3 design (single fused NEFF):
  Phase A streams x then w at fp32, computing exact local amaxes on DVE;
  amax_x / amax_w are AllReduce(max)'d (a dummy warmup AllReduce pays the
  first-collective setup under the x loads). x is PE-transposed as it
  arrives and drained to a RAW fp16 xT (8MiB, no scale needed) so the
  transposes never gate on the ARx result; once sx lands, ACT quantizes
  xT -> 4MiB resident fp8 lhsT codes and xT's space is recycled. The tail
  of the w stream (last RETAIN row-pair tiles in stream order) is
  ACT-downcast to resident fp16 (1MiB/tile), cutting the phase-B re-read
  by 2MiB/tile; the stream order is permuted so the retained set spreads
  across batches b1..b3, balancing phase-B DMA per batch against the PE.
  Phase B re-reads only the non-retained w, quantizes on DVE (fp32 for
  re-read tiles, 2x-rate fp16 for retained), and runs DoubleRow fp8
  matmuls (fp32 PSUM accum) in mt-pair sweeps over 8 PSUM banks, drains
  bias+rescale to bf16 and stores via SWDGE (host upcasts).

Performance model (from ntff profiling):
  - The 16 SDMA engines (~22GB/s each on 8-16KB descriptors) bind phase A
    (80MiB: 16 x + 64 w) and roughly tie the PE in phase B (re-read
    50MiB + 8 out vs ~160us of DoubleRow matmul). All tiles move as
    [128, 2, N] row-pairs (one 16KB-contiguous descriptor per partition).
  - Engine queues are strict FIFO: all load triggers ride the sync HWDGE
    ring; the scalar (ACT) queue holds only the x drains / xqt quants /
    retention downcasts, each gated strictly later than the last, so
    nothing head-of-line blocks. sx math sits SX_DEPTH w-reduces deep in
    the DVE FIFO so DVE reaches it just as the ARx result lands.
  - The ARw collective (~40us against a busy SDMA path) is covered by a
    4-deep re-read prefetch prologue into the freed stage slots.

Quantization math (matches the reference lattice exactly): s' = 224/amax
  (= fl(448/amax)/2 exactly) because TRN fp8_e4m3 tops out at 240, not
  448: the OCP e4m3fn lattice scaled by 1/2 lands exactly on the TRN
  lattice. Matmul runs on raw fp8 codes (exact products, fp32 PSUM
  accum); output is rescaled by c = 1/(sx'*sw'). x codes pass through a
  raw fp16 intermediate and retained w tiles are quantized from fp16:
  the extra 2^-11 rounding flips ~0.8% of codes by 1 ulp, adding ~1e-2
  of the 2e-2 relative budget (measured: comfortably inside the gate).

Per-core HBM traffic: 16 (x) + 64 (w) + 50 (w re-read) + 8 (out bf16)
= 138MiB, one NEFF ramp.
"""

import os
import sys

for _p in ("/root/.axon_site", "/root/.axon_site/_ro/trn_rl_repo", "/opt/trn_rl_repo"):
    if os.path.isdir(_p) and _p not in sys.path:
        sys.path.append(_p)

import numpy as np

import concourse.bass as bass
import concourse.bass_isa as bass_isa
import concourse.mybir as mybir
import concourse.tile as tile
from concourse import bacc
from concourse.bass_utils import run_bass_kernel_spmd
from concourse.masks import make_identity

# Problem shape (hardcoded per contest rules).
B, M, K, N = 32, 512, 2048, 2048
NCORES = 8
BL = B // NCORES          # 4 batch slices per core
P = 128
KT = K // P               # 16 k-tiles per batch
KP = KT // 2              # 8 k-groups (256 rows, row-pair packed) per batch
MT = M // P               # 4 m-tiles
NFREE = 512               # matmul moving free dim (one PSUM bank)
NT = N // NFREE           # 4 n-tiles
SX_DEPTH = 14             # staged (2MiB) w reduces before sx in the DVE FIFO
RETAIN = 7                # w k-group tiles retained as fp16 (with_bias: -2)
PREFETCH = 3              # phase-B re-read loads in flight before 1st quant
FP8_HALF_MAX = 224.0      # 448/2: OCP grid mapped onto TRN e4m3

F32 = mybir.dt.float32
F16 = mybir.dt.float16
BF16 = mybir.dt.bfloat16
FP8 = mybir.dt.float8e4

_cache = {}


def _build_fused_nc(with_bias=True):
    nc = bacc.Bacc("TRN2", target_bir_lowering=False, debug=False, num_devices=NCORES)
    x = nc.dram_tensor("x", [BL, M, K], F32, kind="ExternalInput")
    w = nc.dram_tensor("w", [BL, K, N], F32, kind="ExternalInput")
    bias = nc.dram_tensor("bias", [BL, 1, N], F32, kind="ExternalInput")
    consts = nc.dram_tensor("consts", [1, 2], F32, kind="ExternalInput")
    out = nc.dram_tensor("out", [BL, M, N], BF16, kind="ExternalOutput")

    rg = [list(range(NCORES))]
    retain = RETAIN if with_bias is False else RETAIN - 2

    # w stream order: natural order with the retained set moved to the
    # end so retention only needs SBUF after the xT space frees. The
    # retained set spreads over b1..b3 to even phase-B DMA per batch.
    flat = [(b, t) for b in range(BL) for t in range(KP)]
    # spread across batches so every batch keeps ~38us of phase-B re-read
    # DMA to overlap its PE sweeps (b3-heavy retention leaves a pure-
    # compute tail instead)
    ret_set = [(0, 7), (1, 6), (1, 7), (2, 6),
               (2, 7), (3, 6), (3, 7)][-retain:]
    stream_plan = [bt for bt in flat if bt not in ret_set] + ret_set

    def w_pair_src(b, t):
        """w[b] rows [256t, 256t+256) as [128, 2, N]: partition p holds DRAM
        rows 2p/2p+1 -> one 16KB-contiguous descriptor per partition."""
        return w[b, t * 2 * P:(t + 1) * 2 * P, :].rearrange(
            "(p r) n -> p r n", r=2
        )

    def x_half_src(b, s):
        """x[b] rows [256s, 256s+256) as [128, 2, K]: partition p holds
        rows {256s + p, 256s + 128 + p} (plain m-blocks u = 2s, 2s+1)."""
        return x[b, s * 2 * P:(s + 1) * 2 * P, :].rearrange(
            "(u p) n -> p u n", u=2
        )

    with tile.TileContext(nc) as tc:
        with (
            tc.tile_pool(name="small", bufs=1) as small,
            tc.tile_pool(name="acc", bufs=1) as accp,
            tc.tile_pool(name="xqt", bufs=1) as xqtp,
            tc.tile_pool(name="wstage", bufs=3) as wstage,
            tc.tile_pool(name="dram", bufs=8, space="DRAM") as dram,
        ):
            ident = small.tile([P, P], F32, name="ident")
            make_identity(nc, ident[:])
            cst = small.tile([1, 2], F32, name="cst")
            nc.sync.dma_start(cst[:], consts[0:1, :])
            # scl slots: 0=1/ax, 1=sx, 2=1/aw, 3=sw, 4=sx*sw, 5=c
            scl = small.tile([1, 8], F32, name="scl")
            axg = small.tile([1, 1], F32, name="axg")
            awg = small.tile([1, 1], F32, name="awg")
            awg2 = small.tile([1, 1], F32, name="awg2")
            cb = small.tile([P, 4], F32, name="cb")   # 0=sx, 1=sw, 2=c

            acc = accp.tile([P, 8 + BL * KP], F32, name="acc")
            red = accp.tile([P, 2], F32, name="red")
            par = accp.tile([P, 2], F32, name="par")

            # resident fp8 lhsT codes, batch-major so each batch's quant is
            # one contiguous DVE op: [ki, b, t, par, u*128 + c] with
            # (ki, par) pairing k = 256t + 2*ki + par (matches w pairing)
            # and plain m-blocks m = u*128 + c.
            xqt = xqtp.tile([P, BL, KP, 2, M], FP8, name="xqt")

            dum_in = dram.tile([1, 8], F32, name="dum_in")
            dum_out = dram.tile([1, 8], F32, name="dum_out")
            dum2_in = dram.tile([1, 8], F32, name="dum2_in")
            dum2_out = dram.tile([1, 8], F32, name="dum2_out")
            ar_in = dram.tile([1, 8], F32, name="ar_in")
            ar_out = dram.tile([1, 8], F32, name="ar_out")
            ar2_in = dram.tile([1, 8], F32, name="ar2_in")
            ar2_out = dram.tile([1, 8], F32, name="ar2_out")

            # warmup collective: pays the ~80us first-collective setup while
            # the x loads stream.
            nc.gpsimd.dma_start(dum_in[0:1, 0:2], cst[:])
            nc.gpsimd.collective_compute(
                "AllReduce", mybir.AluOpType.max, replica_groups=rg,
                ins=[dum_in.opt()], outs=[dum_out.opt()],
            )

            # xT lives on the RIGHT SBUF stack so its release (gated on the
            # ARx-dependent xqt quants, which run late, hidden under the
            # ARw collective) never blocks the retention pool, which takes
            # xstage's LEFT-stack space as soon as the transposes finish.
            xtp = tc.alloc_tile_pool(name="xt", bufs=BL, side="right")
            xstage = tc.alloc_tile_pool(name="xstage", bufs=3)
            trps = tc.alloc_tile_pool(name="trps", bufs=6, space="PSUM")

            col = [8]
            wret = {}
            wretp = [None]
            nld = [0]

            def stage_w_load(bt):
                # alternate the two HWDGE rings (sync/scalar): a single
                # ring's in-order completion handling costs ~0.6us/tile.
                # (sync also carries the transpose-gated x loads up front,
                # so early scalar-ring w tiles keep the DMA fed.)
                nld[0] += 1
                eng = nc.sync if nld[0] % 2 == 0 else nc.scalar
                ws = wstage.tile([P, 2, N], F32, name="ws", tag="ws")
                eng.dma_start(ws[:], w_pair_src(*bt))
                nc.vector.tensor_reduce(
                    acc[:, col[0]:col[0] + 1], ws[:],
                    axis=mybir.AxisListType.XY, op=mybir.AluOpType.max,
                    apply_absolute_value=True,
                )
                col[0] += 1
                if bt in ret_set:
                    wr = wretp[0].tile([P, 2, N], F16, name="wr", tag="wr")
                    nc.scalar.activation(
                        wr[:], ws[:], mybir.ActivationFunctionType.Copy,
                    )
                    wret[bt] = wr

            # ---- x: stream, amax, PE-transpose, drain raw fp16 xT ----
            # The x stream is transpose-paced (~20us/batch); w-load blocks
            # interleave between x batches so the stream keeps the DMA
            # engines saturated while x trickles.
            xts = []
            for b in range(BL):
                views = {}
                for s in range(2):
                    st = xstage.tile([P, 2, K], F32, name="xs", tag="xs")
                    nc.sync.dma_start(st[:], x_half_src(b, s))
                    nc.vector.tensor_reduce(
                        acc[:, 2 * b + s:2 * b + s + 1], st[:],
                        axis=mybir.AxisListType.XY, op=mybir.AluOpType.max,
                        apply_absolute_value=True,
                    )
                    for j in range(2):
                        views[2 * s + j] = st[:, j, :].rearrange(
                            "p (k two) -> p two k", two=2
                        )
                xt = xtp.tile([P, KP, 2, M], F16, name="xt", tag="xt")
                for t in range(KP):
                    for parp in range(2):
                        # one single-bank [P, 512] psum per (t, parp):
                        # a 2-bank psum AP puts the ACT drain on a slow
                        # cross-bank read path (~4x)
                        ps = trps.tile([P, M], F32, name="tps", tag="tps")
                        for u in range(MT):
                            nc.tensor.transpose(
                                ps[:, u * P:(u + 1) * P],
                                views[u][:, parp, t * P:(t + 1) * P],
                                ident[:],
                            )
                        nc.scalar.activation(
                            xt[:, t, parp, :], ps[:],
                            mybir.ActivationFunctionType.Copy,
                        )
                xts.append(xt)
                if b < BL - 1:
                    for bt in stream_plan[8 * b:8 * (b + 1)]:
                        stage_w_load(bt)

            # ---- amax_x AllReduce trigger (result consumed later) ----
            nc.vector.tensor_reduce(
                red[:, 0:1], acc[:, 0:2 * BL],
                axis=mybir.AxisListType.X, op=mybir.AluOpType.max,
            )
            nc.gpsimd.partition_all_reduce(
                par[:, 0:1], red[:, 0:1], channels=P,
                reduce_op=bass_isa.ReduceOp.max,
            )
            nc.gpsimd.dma_start(ar_in[0:1, 0:1], par[0:1, 0:1])
            nc.gpsimd.collective_compute(
                "AllReduce", mybir.AluOpType.max, replica_groups=rg,
                ins=[ar_in.opt()], outs=[ar_out.opt()],
            )
            nc.gpsimd.dma_start(axg[:], ar_out[0:1, 0:1])

            trps.release()
            xstage.release()

            # retention pool over xstage's freed space (gated only on the
            # x transposes, NOT on the ARx result — the x amaxes and the
            # collective can land arbitrarily late without stalling the w
            # stream or the retention copies)
            wretp[0] = tc.alloc_tile_pool(name="wret", bufs=max(retain, 1))

            for bt in stream_plan[8 * (BL - 1):8 * (BL - 1) + 4]:
                stage_w_load(bt)

            # ---- w-amax AllReduce, part 1 (stream tiles 0-27) ----
            # Fires ~20us before the stream ends, so its 25-75us exec
            # latency (dominated by inter-core skew) hides under the
            # stream tail; part 2 then runs on a freshly-aligned warm
            # CC mesh, which is consistently fast.
            nc.vector.tensor_reduce(
                red[:, 1:2], acc[:, 8:col[0]],
                axis=mybir.AxisListType.X, op=mybir.AluOpType.max,
            )
            nc.gpsimd.partition_all_reduce(
                par[:, 1:2], red[:, 1:2], channels=P,
                reduce_op=bass_isa.ReduceOp.max,
            )
            nc.gpsimd.dma_start(dum2_in[0:1, 0:1], par[0:1, 1:2])
            nc.gpsimd.collective_compute(
                "AllReduce", mybir.AluOpType.max, replica_groups=rg,
                ins=[dum2_in.opt()], outs=[dum2_out.opt()],
            )
            arw1_cols = col[0]

            for bt in stream_plan[8 * (BL - 1) + 4:]:
                stage_w_load(bt)

            # ---- phase A -> B boundary: w-amax part 2 (last 4 tiles) ----
            nc.vector.tensor_reduce(
                red[:, 0:1], acc[:, arw1_cols:col[0]],
                axis=mybir.AxisListType.X, op=mybir.AluOpType.max,
            )
            # sx = 224 / max(amax_x, 1e-12)
            nc.vector.tensor_scalar_max(axg[:], axg[:], 1e-12)
            nc.vector.reciprocal(scl[0:1, 0:1], axg[:])
            nc.vector.tensor_scalar_mul(scl[0:1, 1:2], scl[0:1, 0:1], FP8_HALF_MAX)
            nc.gpsimd.partition_broadcast(cb[:, 0:1], scl[0:1, 1:2])
            sx_ap = cb[:, 0:1]
            nc.gpsimd.partition_all_reduce(
                par[:, 0:1], red[:, 0:1], channels=P,
                reduce_op=bass_isa.ReduceOp.max,
            )
            nc.gpsimd.dma_start(ar2_in[0:1, 0:1], par[0:1, 0:1])
            nc.gpsimd.collective_compute(
                "AllReduce", mybir.AluOpType.max, replica_groups=rg,
                ins=[ar2_in.opt()], outs=[ar2_out.opt()],
            )
            # xqt quants on DVE while the part-2 collective flies
            for b in range(BL):
                nc.vector.tensor_scalar(
                    xqt[:, b], xts[b][:], sx_ap, None,
                    op0=mybir.AluOpType.mult,
                )
            xtp.release()
            nc.gpsimd.dma_start(awg[:], dum2_out[0:1, 0:1])
            nc.gpsimd.dma_start(awg2[:], ar2_out[0:1, 0:1])
            # sw = 224 / max(amax_w, 1e-12); c = 1/(sx*sw)
            nc.vector.tensor_tensor(
                awg[:], awg[:], awg2[:], mybir.AluOpType.max,
            )
            nc.vector.tensor_scalar_max(awg[:], awg[:], 1e-12)
            nc.vector.reciprocal(scl[0:1, 2:3], awg[:])
            nc.vector.tensor_scalar_mul(scl[0:1, 3:4], scl[0:1, 2:3], FP8_HALF_MAX)
            nc.vector.tensor_tensor(
                scl[0:1, 4:5], scl[0:1, 1:2], scl[0:1, 3:4],
                mybir.AluOpType.mult,
            )
            nc.vector.reciprocal(scl[0:1, 5:6], scl[0:1, 4:5])
            nc.gpsimd.partition_broadcast(cb[:, 1:2], scl[0:1, 3:4])
            nc.gpsimd.partition_broadcast(cb[:, 2:3], scl[0:1, 5:6])
            sw_ap = cb[:, 1:2]
            c_ap = cb[:, 2:3]

            # ---- phase B: software-pipelined re-read + quantize + mm ----
            # Engine split keeps every FIFO stall-free: DVE runs ONLY the
            # re-read quants (so the load pipeline is never queued behind
            # drains at batch boundaries); ACT runs the retained-tile
            # quants (fp16, ready as soon as sw lands) plus all drains.
            # Matmuls sweep u-granular (4 PSUM banks), ping-ponged so the
            # PE never waits on a bank drain.
            wqp = tc.alloc_tile_pool(name="wq", bufs=11)
            ostp = tc.alloc_tile_pool(name="ost", bufs=2)
            if with_bias:
                bias1p = tc.alloc_tile_pool(name="bias1", bufs=1)
                biasbp = tc.alloc_tile_pool(name="biasb", bufs=2)

            reread_plan = [bt for bt in flat if bt not in ret_set]
            stage_tiles = {}
            nload = [0]

            def issue_load():
                # prologue rides sync only (the gpsimd queue may be held
                # by the in-flight ARw collective); the steady pipeline
                # alternates sync/gpsimd (the scalar queue is busy with
                # drains, which would head-of-line block triggers).
                if nload[0] >= len(reread_plan):
                    return
                bt = reread_plan[nload[0]]
                eng = nc.sync if (nload[0] < PREFETCH or nload[0] % 2 == 0) \
                    else nc.gpsimd
                st = wstage.tile([P, 2, N], F32, name="ws", tag="ws")
                eng.dma_start(st[:], w_pair_src(*bt))
                stage_tiles[bt] = st
                nload[0] += 1

            for _ in range(PREFETCH):
                issue_load()

            mmps = tc.alloc_tile_pool(name="mmps", bufs=4, space="PSUM")

            wq_all = {}
            for b_, t_ in flat:
                wqt = wqp.tile([P, 2, N], FP8, name="wq", tag="wq")
                if (b_, t_) in ret_set:
                    if with_bias:
                        nc.vector.tensor_scalar(
                            wqt[:], wret[(b_, t_)][:], sw_ap, None,
                            op0=mybir.AluOpType.mult,
                        )
                    else:
                        nc.scalar.activation(
                            wqt[:], wret[(b_, t_)][:],
                            mybir.ActivationFunctionType.Copy, scale=sw_ap,
                        )
                else:
                    nc.vector.tensor_scalar(
                        wqt[:], stage_tiles.pop((b_, t_))[:], sw_ap, None,
                        op0=mybir.AluOpType.mult,
                    )
                    issue_load()
                wq_all[(b_, t_)] = wqt

                if t_ == KP - 1:
                    b = b_
                    if with_bias:
                        b1 = bias1p.tile([1, N], BF16, name="b1", tag="b1")
                        nc.gpsimd.dma_start(b1[:], bias[b, :, :])
                        bb = biasbp.tile([P, N], BF16, name="bb", tag="bb")
                        nc.gpsimd.partition_broadcast(bb[:], b1[:])

                    wq_tiles = [wq_all.pop((b, t)) for t in range(KP)]
                    ost2 = None
                    for u in range(MT):
                        if u % 2 == 0:
                            ost2 = ostp.tile([P, 2, N], BF16,
                                             name="ost", tag="ost")
                        psums = [
                            mmps.tile([P, 2 * NFREE], F32,
                                      name=f"mm{h}", tag="mm")
                            for h in range(NT // 2)
                        ]
                        for t in range(KP):
                            lhsT = xqt[:, b, t, :, u * P:(u + 1) * P]
                            for nt in range(NT):
                                ps = psums[nt // 2]
                                lo = (nt % 2) * NFREE
                                nc.tensor.matmul(
                                    ps[:, lo:lo + NFREE],
                                    lhsT,
                                    wq_tiles[t][:, :,
                                                nt * NFREE:(nt + 1) * NFREE],
                                    start=(t == 0),
                                    stop=(t == KP - 1),
                                    perf_mode=mybir.MatmulPerfMode.DoubleRow,
                                )
                        for h in range(NT // 2):
                            o_ap = ost2[:, u % 2,
                                        h * 2 * NFREE:(h + 1) * 2 * NFREE]
                            if with_bias:
                                nc.vector.scalar_tensor_tensor(
                                    o_ap, psums[h][:], c_ap,
                                    bb[:, h * 2 * NFREE:(h + 1) * 2 * NFREE],
                                    op0=mybir.AluOpType.mult,
                                    op1=mybir.AluOpType.add,
                                )
                            else:
                                nc.scalar.activation(
                                    o_ap, psums[h][:],
                                    mybir.ActivationFunctionType.Copy,
                                    scale=c_ap,
                                )
                        if u % 2 == 1:
                            # plain m-blocks: m = 256*mh + mi*128 + c; the
                            # store rides the scalar HWDGE ring right after
                            # its drains (sync/gpsimd carry the re-reads).
                            mh = u // 2
                            dst = out[b, 2 * mh * P:(2 * mh + 2) * P,
                                      :].rearrange("(r p) n -> p r n", r=2)
                            nc.scalar.dma_start(dst, ost2[:])

            mmps.release()
            if with_bias:
                biasbp.release()
                bias1p.release()
            ostp.release()
            wqp.release()
            wretp[0].release()

    nc.compile()
    return nc


def _get_nc(with_bias):
    key = "fused_b" if with_bias else "fused_nb"
    if key not in _cache:
        _cache[key] = _build_fused_nc(with_bias)
    return _cache[key]


# test.py introspection: exec times (ns) of the last kernel() call.
last_run_info = {}


def kernel(input, weight, bias, _profile=False, _repeat=1, _trace_kwargs=None):
    input = np.ascontiguousarray(input, dtype=np.float32)
    weight = np.ascontiguousarray(weight, dtype=np.float32)
    bias = np.ascontiguousarray(bias, dtype=np.float32)
    assert input.shape == (B, M, K) and weight.shape == (B, K, N)
    assert bias.shape == (B, 1, N)

    consts = np.array([[FP8_HALF_MAX, 1.0]], dtype=np.float32)
    in_maps = [
        {
            "x": input[c * BL:(c + 1) * BL],
            "w": weight[c * BL:(c + 1) * BL],
            "bias": bias[c * BL:(c + 1) * BL],
            "consts": consts,
        }
        for c in range(NCORES)
    ]

    kw = dict(trace=_profile)
    if _trace_kwargs:
        kw.update(_trace_kwargs)

    # bias is exactly zero in this workload; the no-bias NEFF skips the
    # broadcast-add (drains become scaled copies, ACT-assisted at the tail).
    # The with-bias NEFF stays available for correctness on any input.
    nc = _get_nc(with_bias=bool(np.any(bias)))
    times = []
    res = None
    for _ in range(max(1, _repeat)):
        res = run_bass_kernel_spmd(nc, in_maps, core_ids=list(range(NCORES)), **kw)
        times.append(res.exec_time_ns)

    last_run_info.clear()
    last_run_info["amax_times"] = None
    last_run_info["mm_times"] = times
    last_run_info["amax_exec_ns"] = None
    last_run_info["mm_exec_ns"] = min(t for t in times if t) if any(times) else None
    last_run_info["mm_results"] = res

    out = np.concatenate(
        [np.asarray(res.results[c]["out"]).astype(np.float32) for c in range(NCORES)],
        axis=0,
    )
    return out


# revision 48
# speedup vs baseline: 1.0493x; 1.0107x over previous
"""FP8 batch-matmul-dense kernel for Trainium2 (8 NeuronCores, batch-sharded).

Problem: out[b] = fp8qdq(x)[b] @ fp8qdq(w)[b] + bias[b]
  x: [32, 512, 2048] f32, w: [32, 2048, 2048] f32, bias: [32, 1, 2048] f32
  fp8qdq = torchao-style dynamic tensorwise scaling: s = 448/amax(|t|),
  q = e4m3fn(t*s), dq = q/s. Global (whole-tensor) amax.

Sharding: batch axis across 8 cores, 4 slices each (expert-parallel style).

v3 design (single fused NEFF):
  Phase A streams x then w at fp32, computing exact local amaxes on DVE;
  amax_x / amax_w are AllReduce(max)'d (a dummy warmup AllReduce pays the
  first-collective setup under the x loads). x is PE-transposed as it
  arrives and drained to a RAW fp16 xT (8MiB, no scale needed) so the
  transposes never gate on the ARx result; once sx lands, ACT quantizes
  xT -> 4MiB resident fp8 lhsT codes and xT's space is recycled. The tail
  of the w stream (last RETAIN row-pair tiles in stream order) is
  ACT-downcast to resident fp16 (1MiB/tile), cutting the phase-B re-read
  by 2MiB/tile; the stream order is permuted so the retained set spreads
  across batches b1..b3, balancing phase-B DMA per batch against the PE.
  Phase B re-reads only the non-retained w, quantizes on DVE (fp32 for
  re-read tiles, 2x-rate fp16 for retained), and runs DoubleRow fp8
  matmuls (fp32 PSUM accum) in mt-pair sweeps over 8 PSUM banks, drains
  bias+rescale to bf16 and stores via SWDGE (host upcasts).

Performance model (from ntff profiling):
  - The 16 SDMA engines (~22GB/s each on 8-16KB descriptors) bind phase A
    (80MiB: 16 x + 64 w) and roughly tie the PE in phase B (re-read
    50MiB + 8 out vs ~160us of DoubleRow matmul). All tiles move as
    [128, 2, N] row-pairs (one 16KB-contiguous descriptor per partition).
  - Engine queues are strict FIFO: all load triggers ride the sync HWDGE
    ring; the scalar (ACT) queue holds only the x drains / xqt quants /
    retention downcasts, each gated strictly later than the last, so
    nothing head-of-line blocks. sx math sits SX_DEPTH w-reduces deep in
    the DVE FIFO so DVE reaches it just as the ARx result lands.
  - The ARw collective (~40us against a busy SDMA path) is covered by a
    4-deep re-read prefetch prologue into the freed stage slots.

Quantization math (matches the reference lattice exactly): s' = 224/amax
  (= fl(448/amax)/2 exactly) because TRN fp8_e4m3 tops out at 240, not
  448: the OCP e4m3fn lattice scaled by 1/2 lands exactly on the TRN
  lattice. Matmul runs on raw fp8 codes (exact products, fp32 PSUM
  accum); output is rescaled by c = 1/(sx'*sw'). x codes pass through a
  raw fp16 intermediate and retained w tiles are quantized from fp16:
  the extra 2^-11 rounding flips ~0.8% of codes by 1 ulp, adding ~1e-2
  of the 2e-2 relative budget (measured: comfortably inside the gate).

Per-core HBM traffic: 16 (x) + 64 (w) + 50 (w re-read) + 8 (out bf16)
= 138MiB, one NEFF ramp.
"""

import os
import sys

for _p in ("/root/.axon_site", "/root/.axon_site/_ro/trn_rl_repo", "/opt/trn_rl_repo"):
    if os.path.isdir(_p) and _p not in sys.path:
        sys.path.append(_p)

import numpy as np

import concourse.bass as bass
import concourse.bass_isa as bass_isa
import concourse.mybir as mybir
import concourse.tile as tile
from concourse import bacc
from concourse.bass_utils import run_bass_kernel_spmd
from concourse.masks import make_identity

# Problem shape (hardcoded per contest rules).
B, M, K, N = 32, 512, 2048, 2048
NCORES = 8
BL = B // NCORES          # 4 batch slices per core
P = 128
KT = K // P               # 16 k-tiles per batch
KP = KT // 2              # 8 k-groups (256 rows, row-pair packed) per batch
MT = M // P               # 4 m-tiles
NFREE = 512               # matmul moving free dim (one PSUM bank)
NT = N // NFREE           # 4 n-tiles
SX_DEPTH = 14             # staged (2MiB) w reduces before sx in the DVE FIFO
RETAIN = 7                # w k-group tiles retained as fp16 (with_bias: -2)
PREFETCH = 3              # phase-B re-read loads in flight before 1st quant
FP8_HALF_MAX = 224.0      # 448/2: OCP grid mapped onto TRN e4m3

F32 = mybir.dt.float32
F16 = mybir.dt.float16
BF16 = mybir.dt.bfloat16
FP8 = mybir.dt.float8e4

_cache = {}


def _build_fused_nc(with_bias=True):
    nc = bacc.Bacc("TRN2", target_bir_lowering=False, debug=False, num_devices=NCORES)
    x = nc.dram_tensor("x", [BL, M, K], F32, kind="ExternalInput")
    w = nc.dram_tensor("w", [BL, K, N], F32, kind="ExternalInput")
    bias = nc.dram_tensor("bias", [BL, 1, N], F32, kind="ExternalInput")
    consts = nc.dram_tensor("consts", [1, 2], F32, kind="ExternalInput")
    out = nc.dram_tensor("out", [BL, M, N], BF16, kind="ExternalOutput")

    rg = [list(range(NCORES))]
    retain = RETAIN if with_bias is False else RETAIN - 2

    # w stream order: natural order with the retained set moved to the
    # end so retention only needs SBUF after the xT space frees. The
    # retained set spreads over b1..b3 to even phase-B DMA per batch.
    flat = [(b, t) for b in range(BL) for t in range(KP)]
    # spread across batches so every batch keeps ~38us of phase-B re-read
    # DMA to overlap its PE sweeps (b3-heavy retention leaves a pure-
    # compute tail instead)
    ret_set = [(0, 7), (1, 6), (1, 7), (2, 6),
               (2, 7), (3, 6), (3, 7)][-retain:]
    stream_plan = [bt for bt in flat if bt not in ret_set] + ret_set

    def w_pair_src(b, t):
        """w[b] rows [256t, 256t+256) as [128, 2, N]: partition p holds DRAM
        rows 2p/2p+1 -> one 16KB-contiguous descriptor per partition."""
        return w[b, t * 2 * P:(t + 1) * 2 * P, :].rearrange(
            "(p r) n -> p r n", r=2
        )

    def x_half_src(b, s):
        """x[b] rows [256s, 256s+256) as [128, 2, K]: partition p holds
        rows {256s + p, 256s + 128 + p} (plain m-blocks u = 2s, 2s+1)."""
        return x[b, s * 2 * P:(s + 1) * 2 * P, :].rearrange(
            "(u p) n -> p u n", u=2
        )

    with tile.TileContext(nc) as tc:
        with (
            tc.tile_pool(name="small", bufs=1) as small,
            tc.tile_pool(name="acc", bufs=1) as accp,
            tc.tile_pool(name="xqt", bufs=1) as xqtp,
            tc.tile_pool(name="wstage", bufs=3) as wstage,
            tc.tile_pool(name="dram", bufs=8, space="DRAM") as dram,
        ):
            ident = small.tile([P, P], F32, name="ident")
            make_identity(nc, ident[:])
            cst = small.tile([1, 2], F32, name="cst")
            nc.sync.dma_start(cst[:], consts[0:1, :])
            # scl slots: 0=1/ax, 1=sx, 2=1/aw, 3=sw, 4=sx*sw, 5=c
            scl = small.tile([1, 8], F32, name="scl")
            axg = small.tile([1, 1], F32, name="axg")
            awg = small.tile([1, 1], F32, name="awg")
            awg2 = small.tile([1, 1], F32, name="awg2")
            cb = small.tile([P, 4], F32, name="cb")   # 0=sx, 1=sw, 2=c

            acc = accp.tile([P, 8 + BL * KP], F32, name="acc")
            red = accp.tile([P, 2], F32, name="red")
            par = accp.tile([P, 2], F32, name="par")

            # resident fp8 lhsT codes, batch-major so each batch's quant is
            # one contiguous DVE op: [ki, b, t, par, u*128 + c] with
            # (ki, par) pairing k = 256t + 2*ki + par (matches w pairing)
            # and plain m-blocks m = u*128 + c.
            xqt = xqtp.tile([P, BL, KP, 2, M], FP8, name="xqt")

            dum_in = dram.tile([1, 8], F32, name="dum_in")
            dum_out = dram.tile([1, 8], F32, name="dum_out")
            dum2_in = dram.tile([1, 8], F32, name="dum2_in")
            dum2_out = dram.tile([1, 8], F32, name="dum2_out")
            ar_in = dram.tile([1, 8], F32, name="ar_in")
            ar_out = dram.tile([1, 8], F32, name="ar_out")
            ar2_in = dram.tile([1, 8], F32, name="ar2_in")
            ar2_out = dram.tile([1, 8], F32, name="ar2_out")

            # warmup collective: pays the ~80us first-collective setup while
            # the x loads stream.
            nc.gpsimd.dma_start(dum_in[0:1, 0:2], cst[:])
            nc.gpsimd.collective_compute(
                "AllReduce", mybir.AluOpType.max, replica_groups=rg,
                ins=[dum_in.opt()], outs=[dum_out.opt()],
            )

            # xT lives on the RIGHT SBUF stack so its release (gated on the
            # ARx-dependent xqt quants, which run late, hidden under the
            # ARw collective) never blocks the retention pool, which takes
            # xstage's LEFT-stack space as soon as the transposes finish.
            xtp = tc.alloc_tile_pool(name="xt", bufs=BL, side="right")
            xstage = tc.alloc_tile_pool(name="xstage", bufs=3)
            trps = tc.alloc_tile_pool(name="trps", bufs=6, space="PSUM")

            col = [8]
            wret = {}
            wretp = [None]
            nld = [0]

            def stage_w_load(bt):
                # alternate the two HWDGE rings (sync/scalar): a single
                # ring's in-order completion handling costs ~0.6us/tile.
                # (sync also carries the transpose-gated x loads up front,
                # so early scalar-ring w tiles keep the DMA fed.)
                nld[0] += 1
                eng = nc.sync if nld[0] % 2 == 0 else nc.scalar
                ws = wstage.tile([P, 2, N], F32, name="ws", tag="ws")
                eng.dma_start(ws[:], w_pair_src(*bt))
                nc.vector.tensor_reduce(
                    acc[:, col[0]:col[0] + 1], ws[:],
                    axis=mybir.AxisListType.XY, op=mybir.AluOpType.max,
                    apply_absolute_value=True,
                )
                col[0] += 1
                if bt in ret_set:
                    wr = wretp[0].tile([P, 2, N], F16, name="wr", tag="wr")
                    nc.scalar.activation(
                        wr[:], ws[:], mybir.ActivationFunctionType.Copy,
                    )
                    wret[bt] = wr

            # ---- x: stream, amax, PE-transpose, drain raw fp16 xT ----
            # The x stream is transpose-paced (~20us/batch); w-load blocks
            # interleave between x batches so the stream keeps the DMA
            # engines saturated while x trickles.
            xts = []
            for b in range(BL):
                views = {}
                for s in range(2):
                    st = xstage.tile([P, 2, K], F32, name="xs", tag="xs")
                    nc.sync.dma_start(st[:], x_half_src(b, s))
                    nc.vector.tensor_reduce(
                        acc[:, 2 * b + s:2 * b + s + 1], st[:],
                        axis=mybir.AxisListType.XY, op=mybir.AluOpType.max,
                        apply_absolute_value=True,
                    )
                    for j in range(2):
                        views[2 * s + j] = st[:, j, :].rearrange(
                            "p (k two) -> p two k", two=2
                        )
                xt = xtp.tile([P, KP, 2, M], F16, name="xt", tag="xt")
                for t in range(KP):
                    for parp in range(2):
                        # one single-bank [P, 512] psum per (t, parp):
                        # a 2-bank psum AP puts the ACT drain on a slow
                        # cross-bank read path (~4x)
                        ps = trps.tile([P, M], F32, name="tps", tag="tps")
                        for u in range(MT):
                            nc.tensor.transpose(
                                ps[:, u * P:(u + 1) * P],
                                views[u][:, parp, t * P:(t + 1) * P],
                                ident[:],
                            )
                        nc.scalar.activation(
                            xt[:, t, parp, :], ps[:],
                            mybir.ActivationFunctionType.Copy,
                        )
                xts.append(xt)
                if b < BL - 2:
                    for bt in stream_plan[8 * b:8 * (b + 1)]:
                        stage_w_load(bt)

            # ---- amax_x AllReduce trigger (result consumed later) ----
            # Emitted before the tail w-blocks so the red0 column-reduce
            # sits right behind the x reduces in the DVE FIFO: ARx then
            # clears the CC mesh ~90us before the w-amax collectives
            # need it (otherwise they serialize behind a late ARx).
            nc.vector.tensor_reduce(
                red[:, 0:1], acc[:, 0:2 * BL],
                axis=mybir.AxisListType.X, op=mybir.AluOpType.max,
            )
            nc.gpsimd.partition_all_reduce(
                par[:, 0:1], red[:, 0:1], channels=P,
                reduce_op=bass_isa.ReduceOp.max,
            )
            nc.gpsimd.dma_start(ar_in[0:1, 0:1], par[0:1, 0:1])
            nc.gpsimd.collective_compute(
                "AllReduce", mybir.AluOpType.max, replica_groups=rg,
                ins=[ar_in.opt()], outs=[ar_out.opt()],
            )
            nc.gpsimd.dma_start(axg[:], ar_out[0:1, 0:1])

            trps.release()
            xstage.release()

            # retention pool over xstage's freed space (gated only on the
            # x transposes, NOT on the ARx result — the x amaxes and the
            # collective can land arbitrarily late without stalling the w
            # stream or the retention copies)
            wretp[0] = tc.alloc_tile_pool(name="wret", bufs=max(retain, 1))

            for bt in stream_plan[8 * (BL - 2):8 * (BL - 1) + 4]:
                stage_w_load(bt)

            # ---- w-amax AllReduce, part 1 (stream tiles 0-27) ----
            # Fires ~20us before the stream ends, so its 25-75us exec
            # latency (dominated by inter-core skew) hides under the
            # stream tail; part 2 then runs on a freshly-aligned warm
            # CC mesh, which is consistently fast.
            nc.vector.tensor_reduce(
                red[:, 1:2], acc[:, 8:col[0]],
                axis=mybir.AxisListType.X, op=mybir.AluOpType.max,
            )
            nc.gpsimd.partition_all_reduce(
                par[:, 1:2], red[:, 1:2], channels=P,
                reduce_op=bass_isa.ReduceOp.max,
            )
            nc.gpsimd.dma_start(dum2_in[0:1, 0:1], par[0:1, 1:2])
            nc.gpsimd.collective_compute(
                "AllReduce", mybir.AluOpType.max, replica_groups=rg,
                ins=[dum2_in.opt()], outs=[dum2_out.opt()],
            )
            arw1_cols = col[0]

            for bt in stream_plan[8 * (BL - 1) + 4:]:
                stage_w_load(bt)

            # ---- phase A -> B boundary: w-amax part 2 (last 4 tiles) ----
            nc.vector.tensor_reduce(
                red[:, 0:1], acc[:, arw1_cols:col[0]],
                axis=mybir.AxisListType.X, op=mybir.AluOpType.max,
            )
            # sx = 224 / max(amax_x, 1e-12)
            nc.vector.tensor_scalar_max(axg[:], axg[:], 1e-12)
            nc.vector.reciprocal(scl[0:1, 0:1], axg[:])
            nc.vector.tensor_scalar_mul(scl[0:1, 1:2], scl[0:1, 0:1], FP8_HALF_MAX)
            nc.gpsimd.partition_broadcast(cb[:, 0:1], scl[0:1, 1:2])
            sx_ap = cb[:, 0:1]
            nc.gpsimd.partition_all_reduce(
                par[:, 0:1], red[:, 0:1], channels=P,
                reduce_op=bass_isa.ReduceOp.max,
            )
            nc.gpsimd.dma_start(ar2_in[0:1, 0:1], par[0:1, 0:1])
            nc.gpsimd.collective_compute(
                "AllReduce", mybir.AluOpType.max, replica_groups=rg,
                ins=[ar2_in.opt()], outs=[ar2_out.opt()],
            )
            # xqt quants on DVE while the part-2 collective flies
            for b in range(BL):
                nc.vector.tensor_scalar(
                    xqt[:, b], xts[b][:], sx_ap, None,
                    op0=mybir.AluOpType.mult,
                )
            xtp.release()
            nc.gpsimd.dma_start(awg[:], dum2_out[0:1, 0:1])
            nc.gpsimd.dma_start(awg2[:], ar2_out[0:1, 0:1])
            # sw = 224 / max(amax_w, 1e-12); c = 1/(sx*sw)
            nc.vector.tensor_tensor(
                awg[:], awg[:], awg2[:], mybir.AluOpType.max,
            )
            nc.vector.tensor_scalar_max(awg[:], awg[:], 1e-12)
            nc.vector.reciprocal(scl[0:1, 2:3], awg[:])
            nc.vector.tensor_scalar_mul(scl[0:1, 3:4], scl[0:1, 2:3], FP8_HALF_MAX)
            nc.vector.tensor_tensor(
                scl[0:1, 4:5], scl[0:1, 1:2], scl[0:1, 3:4],
                mybir.AluOpType.mult,
            )
            nc.vector.reciprocal(scl[0:1, 5:6], scl[0:1, 4:5])
            nc.gpsimd.partition_broadcast(cb[:, 1:2], scl[0:1, 3:4])
            nc.gpsimd.partition_broadcast(cb[:, 2:3], scl[0:1, 5:6])
            sw_ap = cb[:, 1:2]
            c_ap = cb[:, 2:3]

            # ---- phase B: software-pipelined re-read + quantize + mm ----
            # Engine split keeps every FIFO stall-free: DVE runs ONLY the
            # re-read quants (so the load pipeline is never queued behind
            # drains at batch boundaries); ACT runs the retained-tile
            # quants (fp16, ready as soon as sw lands) plus all drains.
            # Matmuls sweep u-granular (4 PSUM banks), ping-ponged so the
            # PE never waits on a bank drain.
            wqp = tc.alloc_tile_pool(name="wq", bufs=11)
            ostp = tc.alloc_tile_pool(name="ost", bufs=2)
            if with_bias:
                bias1p = tc.alloc_tile_pool(name="bias1", bufs=1)
                biasbp = tc.alloc_tile_pool(name="biasb", bufs=2)

            reread_plan = [bt for bt in flat if bt not in ret_set]
            stage_tiles = {}
            nload = [0]

            def issue_load():
                # prologue rides sync only (the gpsimd queue may be held
                # by the in-flight ARw collective); the steady pipeline
                # alternates sync/gpsimd (the scalar queue is busy with
                # drains, which would head-of-line block triggers).
                if nload[0] >= len(reread_plan):
                    return
                bt = reread_plan[nload[0]]
                eng = nc.sync if (nload[0] < PREFETCH or nload[0] % 2 == 0) \
                    else nc.gpsimd
                st = wstage.tile([P, 2, N], F32, name="ws", tag="ws")
                eng.dma_start(st[:], w_pair_src(*bt))
                stage_tiles[bt] = st
                nload[0] += 1

            for _ in range(PREFETCH):
                issue_load()

            mmps = tc.alloc_tile_pool(name="mmps", bufs=4, space="PSUM")

            wq_all = {}
            for b_, t_ in flat:
                wqt = wqp.tile([P, 2, N], FP8, name="wq", tag="wq")
                if (b_, t_) in ret_set:
                    if with_bias:
                        nc.vector.tensor_scalar(
                            wqt[:], wret[(b_, t_)][:], sw_ap, None,
                            op0=mybir.AluOpType.mult,
                        )
                    else:
                        nc.scalar.activation(
                            wqt[:], wret[(b_, t_)][:],
                            mybir.ActivationFunctionType.Copy, scale=sw_ap,
                        )
                else:
                    nc.vector.tensor_scalar(
                        wqt[:], stage_tiles.pop((b_, t_))[:], sw_ap, None,
                        op0=mybir.AluOpType.mult,
                    )
                    issue_load()
                wq_all[(b_, t_)] = wqt

                if t_ == KP - 1:
                    b = b_
                    if with_bias:
                        b1 = bias1p.tile([1, N], BF16, name="b1", tag="b1")
                        nc.gpsimd.dma_start(b1[:], bias[b, :, :])
                        bb = biasbp.tile([P, N], BF16, name="bb", tag="bb")
                        nc.gpsimd.partition_broadcast(bb[:], b1[:])

                    wq_tiles = [wq_all.pop((b, t)) for t in range(KP)]
                    ost2 = None
                    for u in range(MT):
                        if u % 2 == 0:
                            ost2 = ostp.tile([P, 2, N], BF16,
                                             name="ost", tag="ost")
                        psums = [
                            mmps.tile([P, 2 * NFREE], F32,
                                      name=f"mm{h}", tag="mm")
                            for h in range(NT // 2)
                        ]
                        for t in range(KP):
                            lhsT = xqt[:, b, t, :, u * P:(u + 1) * P]
                            for nt in range(NT):
                                ps = psums[nt // 2]
                                lo = (nt % 2) * NFREE
                                nc.tensor.matmul(
                                    ps[:, lo:lo + NFREE],
                                    lhsT,
                                    wq_tiles[t][:, :,
                                                nt * NFREE:(nt + 1) * NFREE],
                                    start=(t == 0),
                                    stop=(t == KP - 1),
                                    perf_mode=mybir.MatmulPerfMode.DoubleRow,
                                )
                        for h in range(NT // 2):
                            o_ap = ost2[:, u % 2,
                                        h * 2 * NFREE:(h + 1) * 2 * NFREE]
                            if with_bias:
                                nc.vector.scalar_tensor_tensor(
                                    o_ap, psums[h][:], c_ap,
                                    bb[:, h * 2 * NFREE:(h + 1) * 2 * NFREE],
                                    op0=mybir.AluOpType.mult,
                                    op1=mybir.AluOpType.add,
                                )
                            else:
                                nc.scalar.activation(
                                    o_ap, psums[h][:],
                                    mybir.ActivationFunctionType.Copy,
                                    scale=c_ap,
                                )
                        if u % 2 == 1:
                            # plain m-blocks: m = 256*mh + mi*128 + c; the
                            # store rides the scalar HWDGE ring right after
                            # its drains (sync/gpsimd carry the re-reads).
                            mh = u // 2
                            dst = out[b, 2 * mh * P:(2 * mh + 2) * P,
                                      :].rearrange("(r p) n -> p r n", r=2)
                            nc.scalar.dma_start(dst, ost2[:])

            mmps.release()
            if with_bias:
                biasbp.release()
                bias1p.release()
            ostp.release()
            wqp.release()
            wretp[0].release()

    nc.compile()
    return nc


def _get_nc(with_bias):
    key = "fused_b" if with_bias else "fused_nb"
    if key not in _cache:
        _cache[key] = _build_fused_nc(with_bias)
    return _cache[key]


# test.py introspection: exec times (ns) of the last kernel() call.
last_run_info = {}


def kernel(input, weight, bias, _profile=False, _repeat=1, _trace_kwargs=None):
    input = np.ascontiguousarray(input, dtype=np.float32)
    weight = np.ascontiguousarray(weight, dtype=np.float32)
    bias = np.ascontiguousarray(bias, dtype=np.float32)
    assert input.shape == (B, M, K) and weight.shape == (B, K, N)
    assert bias.shape == (B, 1, N)

    consts = np.array([[FP8_HALF_MAX, 1.0]], dtype=np.float32)
    in_maps = [
        {
            "x": input[c * BL:(c + 1) * BL],
            "w": weight[c * BL:(c + 1) * BL],
            "bias": bias[c * BL:(c + 1) * BL],
            "consts": consts,
        }
        for c in range(NCORES)
    ]

    kw = dict(trace=_profile)
    if _trace_kwargs:
        kw.update(_trace_kwargs)

    # bias is exactly zero in this workload; the no-bias NEFF skips the
    # broadcast-add (drains become scaled copies, ACT-assisted at the tail).
    # The with-bias NEFF stays available for correctness on any input.
    nc = _get_nc(with_bias=bool(np.any(bias)))
    times = []
    res = None
    for _ in range(max(1, _repeat)):
        res = run_bass_kernel_spmd(nc, in_maps, core_ids=list(range(NCORES)), **kw)
        times.append(res.exec_time_ns)

    last_run_info.clear()
    last_run_info["amax_times"] = None
    last_run_info["mm_times"] = times
    last_run_info["amax_exec_ns"] = None
    last_run_info["mm_exec_ns"] = min(t for t in times if t) if any(times) else None
    last_run_info["mm_results"] = res

    out = np.concatenate(
        [np.asarray(res.results[c]["out"]).astype(np.float32) for c in range(NCORES)],
        axis=0,
    )
    return out
